# revision 1
# baseline (speedup 1.0000x reference)
"""Expert-parallel MoE FFN kernel for Trainium2 (Bass/Tile).

Problem: per-expert grouped-GEMM FFN
    y[e] = relu(x[e] @ wi[e]) @ wo[e]
with E=8 experts, x:[E,4096,1024] fp32, wi:[E,1024,4096], wo:[E,4096,1024].
Output: [E*4096, 1024] fp32.

Sharding: expert dim E across the 8 NeuronCores (1 expert per core, no
cross-core communication). Each core runs the same SPMD program on its
expert's slabs.

Per-core dataflow (C=4096 tokens, D=1024 d_model, F=4096 d_ff), processed
in token blocks of CB=1024:
  1. x[cblock] is transposed on the PE (128x128 tiles via identity matmul)
     into xT [d-part, c] layout.
  2. mm1: hT[f, c] = relu(wi.T-tile @ xT) accumulated over d chunks; the
     stationary operand is a wi tile [128d x 128f], the moving operand is
     xT [128d x 512c]. Output lands f-on-partitions, which is exactly the
     layout mm2 needs. ReLU is applied by ScalarE on the PSUM->SBUF copy.
  3. mm2: y[c, d] = hT-tile.T @ wo-tile accumulated over all 32 f chunks,
     one PSUM bank per 128-token tile (8 banks, f-contiguous so the PE
     never waits on DMA).
All matmuls use float32r (fp32 read, FP22 multiply, fp32 accumulate):
1 cycle/row at N=512 - same PE rate as bf16 but ~1e-4 relative error.

Weights are streamed (wi+wo re-read once per token block: 4x32MB), x/y
touched once => ~160MB DMA per core, well under the ~900us of PE work.
"""

import numpy as np

P = 128
E = 8
C = 4096
D_MODEL = 1024
D_FF = 4096
CB = 1024  # token block


def build_bass(C=C, D=D_MODEL, F=D_FF, CB=CB):
    import concourse.bacc as bacc
    import concourse.tile as tile
    from concourse import mybir
    from concourse.masks import make_identity

    f32 = mybir.dt.float32
    f32r = mybir.dt.float32r
    relu = mybir.ActivationFunctionType.Relu

    assert C % CB == 0 and CB % 512 == 0 and D % 512 == 0 and F % P == 0
    NB = C // CB  # token blocks
    DCH = D // P  # d_model chunks (contraction of mm1)
    FCH = F // P  # d_ff chunks (contraction of mm2)
    CT = CB // P  # 128-token tiles per block
    CH = CB // 512  # 512-token halves per block (mm1 moving dim)
    DH = D // 512  # 512-wide d_model slices (mm2 moving dim)

    nc = bacc.Bacc("TRN2", target_bir_lowering=False, debug=False)
    x = nc.dram_tensor("x", [C, D], f32, kind="ExternalInput").ap()
    wi = nc.dram_tensor("wi", [D, F], f32, kind="ExternalInput").ap()
    wo = nc.dram_tensor("wo", [F, D], f32, kind="ExternalInput").ap()
    y = nc.dram_tensor("y", [C, D], f32, kind="ExternalOutput").ap()

    wi_r = wi.rearrange("(ko p) f -> p ko f", p=P)  # [128, DCH, F]

    FSS = 2  # f-chunks per wi superslab (1KB DMA packets)
    assert FCH % FSS == 0

    with tile.TileContext(nc) as tc:
        with (
            tc.tile_pool(name="const", bufs=1) as const_pool,
            tc.tile_pool(name="ht", bufs=1) as ht_pool,
            tc.tile_pool(name="xt", bufs=1) as xt_pool,
            tc.tile_pool(name="xs", bufs=2) as xs_pool,
            tc.tile_pool(name="wi", bufs=3) as wi_pool,
            tc.tile_pool(name="wo", bufs=5) as wo_pool,
            tc.tile_pool(name="ys", bufs=2) as ys_pool,
            tc.tile_pool(name="psum", bufs=8, space="PSUM") as psum_pool,
        ):
            ident = const_pool.tile([P, P], f32)
            make_identity(nc, ident[:])

            def ps_tile():
                return psum_pool.tile([P, 512], f32, tag="ps", name="ps")

            # Warm the PE (HAM clock gate) with dependency-free fp32 matmuls
            # while the first x/wi DMAs are still in flight.
            for _ in range(3):
                pw = ps_tile()
                for w in range(4):
                    nc.tensor.matmul(
                        pw[:, w * P : (w + 1) * P],
                        lhsT=ident[:],
                        rhs=ident[:],
                        start=True,
                        stop=True,
                    )

            def issue_wi(fs):
                """Allocate + DMA one wi superslab (block-independent)."""
                wis = wi_pool.tile([P, DCH, FSS * P], f32r, tag="wi", name="wis")
                nc.sync.dma_start(
                    wis[:],
                    wi_r[:, :, fs * FSS * P : (fs + 1) * FSS * P].bitcast(f32r),
                )
                return wis

            def transpose_block(b):
                """x[block b] -> xT[p_d, ko, c] via PE-transpose."""
                c0 = b * CB
                xT = xt_pool.tile([P, DCH, CB], f32r, tag="xt", name="xT")
                for ct in range(CT):
                    xs = xs_pool.tile([P, D], f32, tag="xs", name="xs")
                    nc.sync.dma_start(
                        xs[:], x[c0 + ct * P : c0 + (ct + 1) * P, :]
                    )
                    for kg in range(DCH // 4):
                        pst = ps_tile()
                        for t in range(4):
                            nc.tensor.transpose(
                                pst[:, t * P : (t + 1) * P],
                                xs[:, (kg * 4 + t) * P : (kg * 4 + t + 1) * P],
                                ident[:],
                            )
                        nc.vector.tensor_copy(
                            xT[:, kg * 4 : (kg + 1) * 4, ct * P : (ct + 1) * P],
                            pst[:].rearrange("p (k c) -> p k c", k=4),
                        )
                return xT

            xT = transpose_block(0)
            NSS = FCH // FSS  # wi superslabs per block
            NKEEP = min(3, NSS)  # tail slabs still resident for the next block
            wis_cache = {}
            for b in range(NB):
                c0 = b * CB

                # --- mm1: hT[f, c] = relu(x @ wi)^T for this block ---
                # Alternate the f direction per block: the last NKEEP wi
                # superslabs of block b are still in their pool slots, so
                # block b+1 starts on them with no DMA at all, giving the
                # wi stream a head start instead of a boundary stall.
                fs_order = list(range(NSS)) if b % 2 == 0 else list(
                    range(NSS - 1, -1, -1)
                )
                hT = ht_pool.tile([P, FCH, CB], f32r, tag="ht")
                for fs in fs_order:
                    if fs in wis_cache:
                        wis = wis_cache.pop(fs)
                    else:
                        wis = issue_wi(fs)
                    if b + 1 < NB:
                        wis_cache[fs] = wis
                        if len(wis_cache) > NKEEP:
                            wis_cache.pop(next(iter(wis_cache)))
                    for fi in range(FSS):
                        f = fs * FSS + fi
                        for ch in range(CH):
                            ph = ps_tile()
                            for ko in range(DCH):
                                nc.tensor.matmul(
                                    ph[:],
                                    lhsT=wis[:, ko, fi * P : (fi + 1) * P],
                                    rhs=xT[:, ko, ch * 512 : (ch + 1) * 512],
                                    start=(ko == 0),
                                    stop=(ko == DCH - 1),
                                )
                            nc.scalar.activation(
                                hT[:, f, ch * 512 : (ch + 1) * 512], ph[:], relu
                            )

                # --- mm2: y[c, d] = hT.T @ wo, f-contiguous accumulation ---
                # Next block's x-transposes are emitted between the dh passes
                # so they overlap mm2 instead of stalling the block boundary.
                def issue_wo(f, dh):
                    wos = wo_pool.tile([P, 512], f32r, tag="wo", name="wos")
                    nc.sync.dma_start(
                        wos[:],
                        wo[
                            f * P : (f + 1) * P, dh * 512 : (dh + 1) * 512
                        ].bitcast(f32r),
                    )
                    return wos

                # On the last dh pass the final TAILF f-chunks run ct-major so
                # PSUM banks retire one by one and the next block's mm1 can
                # claim them early.
                TAILF = 4
                for dh in range(DH):
                    if dh == DH - 1 and b + 1 < NB:
                        xT = transpose_block(b + 1)
                    tailf = TAILF if (dh == DH - 1 and FCH > TAILF) else 0
                    pys = [ps_tile() for _ in range(CT)]
                    for f in range(FCH - tailf):
                        wos = issue_wo(f, dh)
                        for ct in range(CT):
                            nc.tensor.matmul(
                                pys[ct][:],
                                lhsT=hT[:, f, ct * P : (ct + 1) * P],
                                rhs=wos[:],
                                start=(f == 0),
                                stop=(f == FCH - 1),
                            )
                    def flush_bank(ct):
                        ysb = ys_pool.tile([P, 512], f32, tag="ys", name="ysb")
                        if ct % 2 == 0:
                            nc.scalar.copy(ysb[:], pys[ct][:])
                        else:
                            nc.vector.tensor_copy(ysb[:], pys[ct][:])
                        nc.sync.dma_start(
                            y[
                                c0 + ct * P : c0 + (ct + 1) * P,
                                dh * 512 : (dh + 1) * 512,
                            ],
                            ysb[:],
                        )

                    if tailf:
                        wos_tail = [issue_wo(f, dh) for f in range(FCH - tailf, FCH)]
                        for ct in range(CT):
                            for k, f in enumerate(range(FCH - tailf, FCH)):
                                nc.tensor.matmul(
                                    pys[ct][:],
                                    lhsT=hT[:, f, ct * P : (ct + 1) * P],
                                    rhs=wos_tail[k][:],
                                    start=False,
                                    stop=(f == FCH - 1),
                                )
                            flush_bank(ct)
                    else:
                        for ct in range(CT):
                            flush_bank(ct)

    nc.compile()
    return nc


_NC_CACHE = {}


def _get_nc(shape_key):
    if shape_key not in _NC_CACHE:
        _NC_CACHE[shape_key] = build_bass(*shape_key)
    return _NC_CACHE[shape_key]


def kernel(dispatched_states, fused_wi_weight, fused_wo_weight):
    from concourse.bass_utils import run_bass_kernel_spmd

    xs = np.ascontiguousarray(np.asarray(dispatched_states, dtype=np.float32))
    wis = np.ascontiguousarray(np.asarray(fused_wi_weight, dtype=np.float32))
    wos = np.ascontiguousarray(np.asarray(fused_wo_weight, dtype=np.float32))
    e, c, d = xs.shape
    f = wis.shape[2]
    assert (e, c, d, f) == (E, C, D_MODEL, D_FF), (e, c, d, f)

    nc = _get_nc((c, d, f, CB))
    in_maps = [{"x": xs[i], "wi": wis[i], "wo": wos[i]} for i in range(e)]
    res = run_bass_kernel_spmd(nc, in_maps, core_ids=list(range(e)))
    out = np.concatenate([res.results[i]["y"] for i in range(e)], axis=0)
    return out.astype(np.float32)



# revision 4
# speedup vs baseline: 1.3817x; 1.3817x over previous
"""Expert-parallel MoE FFN kernel for Trainium2 (Bass/Tile), bf16 edition.

Problem: per-expert grouped-GEMM FFN
    y[e] = relu(x[e] @ wi[e]) @ wo[e]
with E=8 experts, x:[E,4096,1024], wi:[E,1024,4096], wo:[E,4096,1024] (fp32).
Output: [E*4096, 1024] fp32.

Sharding: expert dim E across the 8 NeuronCores (1 expert per core, no
cross-core communication).

Strategy vs the fp32r v1 (1086us):
  * All inputs are converted to bf16 on the HOST (tolerance is 2e-2; bf16
    in / fp32-PSUM accumulate lands ~2e-3). PE rate is identical for bf16
    and fp32r (1 elem/cell/cycle), but bf16:
      - halves all input DMA traffic,
      - lets wi AND wo live in SBUF for the whole kernel (64KB/part each)
        -> zero weight re-streaming, no mm2 DMA dependency at all,
      - x is transposed by the DMA XBAR (2-byte dtype path) instead of the
        PE -> removes ~256 PE transposes (~70us of PE time),
      - enables fast weight load (FWL) for the per-MM LDWEIGHTS.
  * The PE instruction stream is nothing but 4096 N=512 matmuls
    (2048 mm1 + 2048 mm2) ~ 213.5ns each ~ 875us, the bf16-rate roofline.

Per-core dataflow (C=4096 tokens, D=1024 d_model, F=4096 d_ff), token
blocks of CB=512:
  mm1: hT[f,c] = relu(x @ wi)^T : lhsT = wi-tile [128d,128f] (stationary),
       rhs = xT [128d, 512c] (moving), accumulate 8 d-chunks in PSUM fp32,
       ScalarE applies ReLU on the PSUM->SBUF copy, emitting bf16 hT.
  mm2: y[c,d] = hT^T @ wo : lhsT = hT-tile [128f,128c], rhs = wo-slab
       [128f, 512d] from resident wo_sb, 32 f-chunks accumulated per PSUM
       bank, 4 banks (one per 128-token tile), VectorE evacuates fp32 y.
"""

import numpy as np

P = 128
E = 8
C = 4096
D_MODEL = 1024
D_FF = 4096
CB = 512  # token block


def build_bass(C=C, D=D_MODEL, F=D_FF, CB=CB):
    import concourse.bacc as bacc
    import concourse.tile as tile
    from concourse import mybir
    from concourse.masks import make_identity

    f32 = mybir.dt.float32
    bf16 = mybir.dt.bfloat16
    relu = mybir.ActivationFunctionType.Relu

    assert C % CB == 0 and CB % P == 0 and D % 512 == 0 and F % P == 0
    NB = C // CB  # token blocks
    DCH = D // P  # d_model chunks (contraction of mm1)
    FCH = F // P  # d_ff chunks (contraction of mm2)
    CT = CB // P  # 128-token tiles per block
    DH = D // 512  # 512-wide d_model slices (mm2 moving dim)
    assert CB == 512, "mm1 moving operand is one 512-wide chunk per block"

    nc = bacc.Bacc("TRN2", target_bir_lowering=False, debug=False)
    x = nc.dram_tensor("x", [C, D], bf16, kind="ExternalInput").ap()
    wi = nc.dram_tensor("wi", [D, F], bf16, kind="ExternalInput").ap()
    wo = nc.dram_tensor("wo", [F, D], bf16, kind="ExternalInput").ap()
    y = nc.dram_tensor("y", [C, D], f32, kind="ExternalOutput").ap()

    wi_r = wi.rearrange("(ko p) f -> p ko f", p=P)  # [128, DCH, F]
    wo_r = wo.rearrange("(fc p) d -> p fc d", p=P)  # [128, FCH, D]

    with tile.TileContext(nc) as tc:
        with (
            tc.tile_pool(name="const", bufs=1) as const_pool,
            tc.tile_pool(name="wres", bufs=1) as wres_pool,
            tc.tile_pool(name="xt", bufs=2) as xt_pool,
            tc.tile_pool(name="ht", bufs=1) as ht_pool,
            tc.tile_pool(name="ys", bufs=4) as ys_pool,
            tc.tile_pool(name="psum", bufs=8, space="PSUM") as psum_pool,
        ):
            ident = const_pool.tile([P, P], f32)
            make_identity(nc, ident[:])

            # Resident weights: wi as [128d, ko, f], wo as [128f, fc, d].
            wi_sb = wres_pool.tile([P, DCH, F], bf16, name="wi_sb")
            wo_sb = wres_pool.tile([P, FCH, D], bf16, name="wo_sb")

            def load_wi_chunk(wc, WIC):
                nc.sync.dma_start(
                    wi_sb[:, :, wc * WIC : (wc + 1) * WIC],
                    wi_r[:, :, wc * WIC : (wc + 1) * WIC],
                )

            def load_wo_chunk(wc, WOC):
                nc.sync.dma_start(
                    wo_sb[:, wc * WOC : (wc + 1) * WOC, :],
                    wo_r[:, wc * WOC : (wc + 1) * WOC, :],
                )

            def ps_tile():
                return psum_pool.tile([P, 512], f32, tag="ps", name="ps")

            # Warm the PE (HAM clock gate) with dependency-free matmuls
            # while the first wi/x DMAs are in flight.
            for _ in range(3):
                pw = ps_tile()
                for w in range(4):
                    nc.tensor.matmul(
                        pw[:, w * P : (w + 1) * P],
                        lhsT=ident[:],
                        rhs=ident[:],
                        start=True,
                        stop=True,
                    )

            def transpose_load(b):
                """DMA-XBAR transpose x[block b] -> xT [128d, ko, c]."""
                c0 = b * CB
                xT = xt_pool.tile([P, DCH, CB], bf16, tag="xt", name="xT")
                for ko in range(DCH):
                    nc.sync.dma_start(
                        xT[:, ko, :],
                        x[c0 : c0 + CB, ko * P : (ko + 1) * P],
                        transpose=True,
                    )
                return xT

            # DMA emission order = per-queue execution order: first wi
            # chunk, then x block 0, then the rest of wi, then wo (only
            # needed ~55us in when mm2 of block 0 starts).
            load_wi_chunk(0, F // 8)
            xT = transpose_load(0)
            for wc in range(1, 8):
                load_wi_chunk(wc, F // 8)
            for wc in range(4):
                load_wo_chunk(wc, FCH // 4)
            for b in range(NB):
                c0 = b * CB

                # --- mm1: hT[f, c] = relu(x @ wi)^T for this block ---
                hT = ht_pool.tile([P, FCH, CB], bf16, tag="ht", name="hT")
                for fc in range(FCH):
                    ph = ps_tile()
                    for ko in range(DCH):
                        nc.tensor.matmul(
                            ph[:],
                            lhsT=wi_sb[:, ko, fc * P : (fc + 1) * P],
                            rhs=xT[:, ko, :],
                            start=(ko == 0),
                            stop=(ko == DCH - 1),
                        )
                    nc.scalar.activation(hT[:, fc, :], ph[:], relu)

                # Prefetch next block's xT while mm2 runs.
                if b + 1 < NB:
                    xT = transpose_load(b + 1)

                # --- mm2: y[c, d] = hT^T @ wo, f-contiguous accumulation ---
                for dh in range(DH):
                    pys = [ps_tile() for _ in range(CT)]
                    for fc in range(FCH):
                        rhs = wo_sb[:, fc, dh * 512 : (dh + 1) * 512]
                        for ct in range(CT):
                            nc.tensor.matmul(
                                pys[ct][:],
                                lhsT=hT[:, fc, ct * P : (ct + 1) * P],
                                rhs=rhs,
                                start=(fc == 0),
                                stop=(fc == FCH - 1),
                            )
                    for ct in range(CT):
                        ysb = ys_pool.tile([P, 512], f32, tag="ys", name="ysb")
                        nc.vector.tensor_copy(ysb[:], pys[ct][:])
                        nc.sync.dma_start(
                            y[
                                c0 + ct * P : c0 + (ct + 1) * P,
                                dh * 512 : (dh + 1) * 512,
                            ],
                            ysb[:],
                        )

    nc.compile()
    return nc


_NC_CACHE = {}


def _get_nc(shape_key):
    if shape_key not in _NC_CACHE:
        _NC_CACHE[shape_key] = build_bass(*shape_key)
    return _NC_CACHE[shape_key]


def prep_inputs(dispatched_states, fused_wi_weight, fused_wo_weight):
    """Host-side prep: split experts across cores, cast to bf16."""
    import ml_dtypes

    bf = ml_dtypes.bfloat16
    xs = np.asarray(dispatched_states)
    wis = np.asarray(fused_wi_weight)
    wos = np.asarray(fused_wo_weight)
    e, c, d = xs.shape
    f = wis.shape[2]
    assert (e, c, d, f) == (E, C, D_MODEL, D_FF), (e, c, d, f)
    return [
        {
            "x": np.ascontiguousarray(xs[i]).astype(bf),
            "wi": np.ascontiguousarray(wis[i]).astype(bf),
            "wo": np.ascontiguousarray(wos[i]).astype(bf),
        }
        for i in range(e)
    ]


def kernel(dispatched_states, fused_wi_weight, fused_wo_weight):
    from concourse.bass_utils import run_bass_kernel_spmd

    in_maps = prep_inputs(dispatched_states, fused_wi_weight, fused_wo_weight)
    nc = _get_nc((C, D_MODEL, D_FF, CB))
    res = run_bass_kernel_spmd(nc, in_maps, core_ids=list(range(E)))
    out = np.concatenate([res.results[i]["y"] for i in range(E)], axis=0)
    return out.astype(np.float32)
